# revision 6
# baseline (speedup 1.0000x reference)
"""Bass/Trainium2 kernel for nn_GroundingLoss (symmetric token-level InfoNCE).

Math (matches the jax reference exactly):
    sim[a,b,i,j] = sum_k x[a,i,k] * z[b,j,k]
    S[a,b]       = (1/J) * sum_j  [ sum_i softmax_i(sim[a,b,:,j]) * sim[a,b,:,j] ]
    loss         = mean( logsumexp_a(S) - diag + logsumexp_b(S) - diag )

Sharding: the batch axis of x (a) is split across the 8 cores; every core
computes S[a_local, :] against all of z.

Device layout per core (v6): partitions = (b4, j32) per (b,j)-tile (64 tiles
of 128), free = (i, a) with i major, so the softmax i-reduction sits on the
FREE axis and the PE only does the single sim pass.  Tiles run in PAIRS
(PSUM holds two double-buffered [128,2048]f32 groups).  Per pair:
  PE   8 matmuls (K=256 as 2 accum halves, weights reused across i-halves)
  ACT  e  = exp(sim - SHIFT)        (sole exp engine; ~2.4us/pair floor)
  DVE  es = e * sim                 (sole PSUM-capable vector engine)
       plus the es i-half fold (bf16 tensor_tensor runs at 2 elem/cycle)
  Pool the e i-half fold            (Pool is SBUF-only and slow, ~2.4ns/elem
       measured, so it only gets this one contiguous bf16 add)
The ees buffer is laid out (ih, e|es, pt, il*a) so every reduction operand
is a contiguous run; ACT/DVE write their outputs through transposed-dim APs
into that layout.  The half-folded [T,pt,il16,a32] l1 tiles ship to the
host as bf16 (16MB/core, ~50us of DMA split across the SP and ACT hwdge
queues, under the HBM roof); the host folds il16, divides num/den, averages
over j, and runs the tiny [256,256] logsumexp epilogue (softmax weights are
shift-invariant, so no SHIFT correction is needed).
"""

import numpy as np

N, I, J, K = 256, 32, 32, 256
NCORES = 8
NL = N // NCORES          # 32 local a's per core
AF = NL * I               # 1024 rhs cols per K-half (i, a) i-major
BJ = N * J                # 8192 (b, j) pairs
NT = BJ // 128            # 64 (b,j)-tiles of 128 partitions
NP = NT // 2              # 32 tile-pairs
SHIFT = 60.0              # exp shift: safe for |sim| up to ~130

_cached = None


def _build():
    import concourse.bacc as bacc
    import concourse.mybir as mybir
    import concourse.tile as tile

    f32 = mybir.dt.float32
    bf16 = mybir.dt.bfloat16
    AF_T = mybir.ActivationFunctionType

    nc = bacc.Bacc("TRN2", target_bir_lowering=False, debug=False)
    xt_d = nc.dram_tensor("xt", [128, 2 * AF], bf16, kind="ExternalInput").ap()
    zt_d = nc.dram_tensor("zt", [128, 2 * BJ], bf16, kind="ExternalInput").ap()
    out_d = nc.dram_tensor("out", [128, NP, 2, 2, 512], bf16, kind="ExternalOutput").ap()

    with tile.TileContext(nc) as tc:
        with (
            tc.tile_pool(name="const", bufs=1) as cpool,
            tc.tile_pool(name="psum", bufs=2, space="PSUM") as ppool,
            tc.tile_pool(name="ees", bufs=3) as epool,
            tc.tile_pool(name="l1", bufs=3) as lpool,
        ):
            bias_t = cpool.tile([128, 1], f32)
            nc.gpsimd.memset(bias_t[:], -SHIFT)
            xt = cpool.tile([128, 2 * AF], bf16)
            zt = cpool.tile([128, 2 * BJ], bf16)
            # input loads on the ACT hwdge queue; interleave the K-halves so
            # early tiles are ready fast
            nc.scalar.dma_start(xt[:], xt_d[:, :])
            nq = 8
            for q in range(nq):
                for kc in range(2):
                    sl = slice(kc * BJ + q * (BJ // nq), kc * BJ + (q + 1) * (BJ // nq))
                    nc.scalar.dma_start(zt[:, sl], zt_d[:, sl])

            for pr in range(NP):
                sim = ppool.tile([128, 2, 2, 512], f32, tag="sim")  # (pt, ih, il*a)
                for pt in range(2):
                    t = 2 * pr + pt
                    for kc in range(2):
                        lhsT = zt[:, kc * BJ + t * 128 : kc * BJ + (t + 1) * 128]
                        for ih in range(2):
                            nc.tensor.matmul(
                                sim[:, pt, ih],
                                lhsT,
                                xt[:, kc * AF + ih * 512 : kc * AF + (ih + 1) * 512],
                                start=(kc == 0),
                                stop=(kc == 1),
                            )
                # (ih, e|es, pt, il*a): both i-halves of either tensor are
                # contiguous 2048-runs for the folds
                ees = epool.tile([128, 2, 2, 2, 512], bf16, tag="ees")
                e_v = ees[:, :, 0].transpose([0, 2, 1, 3])   # [p, pt, ih, .]
                es_v = ees[:, :, 1].transpose([0, 2, 1, 3])
                nc.scalar.activation(e_v, sim[:], AF_T.Exp, bias=bias_t[:], scale=1.0)
                nc.vector.tensor_mul(es_v, e_v, sim[:])
                # fold the i-halves (i 32 -> 16); host finishes the reduction
                l1 = lpool.tile([128, 2, 2, 512], bf16, tag="l1")  # (e|es, pt, .)
                nc.gpsimd.tensor_add(l1[:, 0], ees[:, 0, 0], ees[:, 1, 0])
                nc.vector.tensor_add(l1[:, 1], ees[:, 0, 1], ees[:, 1, 1])
                # ship; alternate the two hwdge queues
                eng = nc.sync if pr % 2 == 0 else nc.scalar
                eng.dma_start(out_d[:, pr], l1[:])
    nc.compile()
    return nc


def _prep_inputs(x, z):
    import ml_dtypes

    bf = ml_dtypes.bfloat16
    x = np.ascontiguousarray(x, dtype=np.float32).astype(bf)
    z = np.ascontiguousarray(z, dtype=np.float32).astype(bf)
    # zt[p, kc*BJ + b*J + j] = z[b, j, kc*128 + p]
    zt = z.transpose(2, 0, 1).reshape(K, BJ)
    zt = np.concatenate([zt[0:128], zt[128:256]], axis=1)
    zt = np.ascontiguousarray(zt)
    in_maps = []
    for d in range(NCORES):
        xl = x[d * NL : (d + 1) * NL]                  # [a, i, K]
        xt = xl.transpose(2, 1, 0).reshape(K, AF)      # [K, (i, a)]
        xt = np.concatenate([xt[0:128], xt[128:256]], axis=1)
        in_maps.append({"xt": np.ascontiguousarray(xt), "zt": zt})
    return in_maps


def _epilogue(results):
    S = np.empty((N, N), dtype=np.float64)
    for d in range(NCORES):
        # [p=(b4,j), pr, e|es, pt, il16, a]
        arr = results[d]["out"].astype(np.float32).reshape(128, NP, 2, 2, 16, NL)
        nd = arr.sum(axis=4)                           # [p, pr, e|es, pt, a]
        r = nd[:, :, 1] / nd[:, :, 0]                  # [(b4,j), pr, pt, a]
        r = r.reshape(4, J, NT, NL).mean(axis=1).astype(np.float64)  # [b4, t, a]
        S[d * NL : (d + 1) * NL, :] = r.transpose(2, 1, 0).reshape(NL, N)
    diag = np.diagonal(S)
    m0 = S.max(axis=0)
    lx = m0 + np.log(np.exp(S - m0[None, :]).sum(axis=0)) - diag
    m1 = S.max(axis=1)
    lz = m1 + np.log(np.exp(S - m1[:, None]).sum(axis=1)) - diag
    loss = (lx + lz).mean()
    return np.asarray(loss, dtype=np.float32)


def run_on_device(x, z, trace=False):
    """Returns (loss, BassKernelResults)."""
    from concourse.bass_utils import run_bass_kernel_spmd

    global _cached
    if _cached is None:
        _cached = _build()
    nc = _cached
    in_maps = _prep_inputs(x, z)
    res = run_bass_kernel_spmd(nc, in_maps, list(range(NCORES)), trace=trace)
    return _epilogue(res.results), res


def kernel(x, z):
    loss, _ = run_on_device(x, z)
    return loss


# revision 7
# speedup vs baseline: 1.3909x; 1.3909x over previous
"""Bass/Trainium2 kernel for nn_GroundingLoss (symmetric token-level InfoNCE).

Math (matches the jax reference exactly):
    sim[a,b,i,j] = sum_k x[a,i,k] * z[b,j,k]
    S[a,b]       = (1/J) * sum_j  [ sum_i softmax_i(sim[a,b,:,j]) * sim[a,b,:,j] ]
    loss         = mean( logsumexp_a(S) - diag + logsumexp_b(S) - diag )

Sharding: the batch axis of x (a) is split across the 8 cores; every core
computes S[a_local, :] against all of z.

Device layout per core (v7): partitions = (b4, j32) per (b,j)-tile (64 tiles
of 128), free = (i, a) with i major, so the softmax i-reduction sits on the
FREE axis and the PE only does the single sim pass.  Single-tile pipeline
stages with PSUM bufs=4 — the PE->ACT->DVE chain needs depth >= 3 or the PE
stalls waiting for PSUM (measured 6us stalls at depth 2).  Per tile:
  PE   4 matmuls [128,512] (K=256 as 2 accum halves, weights reused)
  ACT  e  = exp(sim - SHIFT)     (sole exp engine)
  DVE  es = e * sim              (sole PSUM-capable vector engine; 1024
       elems at ~1.3ns/elem makes this the ~88us floor of the kernel)
  Pool l1e = e i-half fold       (Pool is slow, ~2.5ns/elem, SBUF-only;
       this 512-elem add is all it can afford)
es ships RAW (bf16, 2KB/part/tile) and e ships half-folded (1KB/part/tile),
both on the otherwise-idle SP hwdge queue (~76us, under the span; 28MB HBM
out of ~330GB/s).  Input loads ride the ACT hwdge queue up front.  The host
does the remaining folds in fp32, divides num/den, averages over j, and
runs the tiny [256,256] logsumexp epilogue (softmax weights are
shift-invariant, so no SHIFT correction is needed).
"""

import numpy as np

N, I, J, K = 256, 32, 32, 256
NCORES = 8
NL = N // NCORES          # 32 local a's per core
AF = NL * I               # 1024 rhs cols per K-half (i, a) i-major
BJ = N * J                # 8192 (b, j) pairs
NT = BJ // 128            # 64 (b,j)-tiles of 128 partitions
SHIFT = 60.0              # exp shift: safe for |sim| up to ~130

_cached = None


def _build():
    import concourse.bacc as bacc
    import concourse.mybir as mybir
    import concourse.tile as tile

    f32 = mybir.dt.float32
    bf16 = mybir.dt.bfloat16
    AF_T = mybir.ActivationFunctionType

    nc = bacc.Bacc("TRN2", target_bir_lowering=False, debug=False)
    xt_d = nc.dram_tensor("xt", [128, 2 * AF], bf16, kind="ExternalInput").ap()
    zt_d = nc.dram_tensor("zt", [128, 2 * BJ], bf16, kind="ExternalInput").ap()
    es_d = nc.dram_tensor("es", [128, NT, 2, 512], bf16, kind="ExternalOutput").ap()
    le_d = nc.dram_tensor("le", [128, NT, 512], bf16, kind="ExternalOutput").ap()

    with tile.TileContext(nc) as tc:
        with (
            tc.tile_pool(name="const", bufs=1) as cpool,
            tc.tile_pool(name="psum", bufs=4, space="PSUM") as ppool,
            tc.tile_pool(name="ees", bufs=4) as epool,
            tc.tile_pool(name="l1", bufs=4) as lpool,
        ):
            bias_t = cpool.tile([128, 1], f32)
            nc.gpsimd.memset(bias_t[:], -SHIFT)
            xt = cpool.tile([128, 2 * AF], bf16)
            zt = cpool.tile([128, 2 * BJ], bf16)
            # input loads on the ACT hwdge queue (SP carries the output
            # stream); interleave the K-halves so early tiles are ready fast
            nc.scalar.dma_start(xt[:], xt_d[:, :])
            nq = 8
            for q in range(nq):
                for kc in range(2):
                    sl = slice(kc * BJ + q * (BJ // nq), kc * BJ + (q + 1) * (BJ // nq))
                    nc.scalar.dma_start(zt[:, sl], zt_d[:, sl])

            for t in range(NT):
                sim = ppool.tile([128, 2, 512], f32, tag="sim")  # (ih, il*a)
                for kc in range(2):
                    lhsT = zt[:, kc * BJ + t * 128 : kc * BJ + (t + 1) * 128]
                    for ih in range(2):
                        nc.tensor.matmul(
                            sim[:, ih],
                            lhsT,
                            xt[:, kc * AF + ih * 512 : kc * AF + (ih + 1) * 512],
                            start=(kc == 0),
                            stop=(kc == 1),
                        )
                ees = epool.tile([128, 2, 2, 512], bf16, tag="ees")  # (e|es, ih, .)
                nc.scalar.activation(ees[:, 0], sim[:], AF_T.Exp, bias=bias_t[:], scale=1.0)
                nc.vector.tensor_mul(ees[:, 1], ees[:, 0], sim[:])
                l1e = lpool.tile([128, 512], bf16, tag="l1e")
                nc.gpsimd.tensor_add(l1e[:], ees[:, 0, 0], ees[:, 0, 1])
                nc.sync.dma_start(es_d[:, t], ees[:, 1])
                nc.sync.dma_start(le_d[:, t], l1e[:])
    nc.compile()
    return nc


def _prep_inputs(x, z):
    import ml_dtypes

    bf = ml_dtypes.bfloat16
    x = np.ascontiguousarray(x, dtype=np.float32).astype(bf)
    z = np.ascontiguousarray(z, dtype=np.float32).astype(bf)
    # zt[p, kc*BJ + b*J + j] = z[b, j, kc*128 + p]
    zt = z.transpose(2, 0, 1).reshape(K, BJ)
    zt = np.concatenate([zt[0:128], zt[128:256]], axis=1)
    zt = np.ascontiguousarray(zt)
    in_maps = []
    for d in range(NCORES):
        xl = x[d * NL : (d + 1) * NL]                  # [a, i, K]
        xt = xl.transpose(2, 1, 0).reshape(K, AF)      # [K, (i, a)]
        xt = np.concatenate([xt[0:128], xt[128:256]], axis=1)
        in_maps.append({"xt": np.ascontiguousarray(xt), "zt": zt})
    return in_maps


def _epilogue(results):
    S = np.empty((N, N), dtype=np.float64)
    for d in range(NCORES):
        es = results[d]["es"].astype(np.float32).reshape(128, NT, I, NL)
        num = es.sum(axis=2)                           # [(b4,j), t, a]
        le = results[d]["le"].astype(np.float32).reshape(128, NT, 16, NL)
        den = le.sum(axis=2)                           # [(b4,j), t, a]
        r = num / den
        r = r.reshape(4, J, NT, NL).mean(axis=1).astype(np.float64)  # [b4, t, a]
        S[d * NL : (d + 1) * NL, :] = r.transpose(2, 1, 0).reshape(NL, N)
    diag = np.diagonal(S)
    m0 = S.max(axis=0)
    lx = m0 + np.log(np.exp(S - m0[None, :]).sum(axis=0)) - diag
    m1 = S.max(axis=1)
    lz = m1 + np.log(np.exp(S - m1[:, None]).sum(axis=1)) - diag
    loss = (lx + lz).mean()
    return np.asarray(loss, dtype=np.float32)


def run_on_device(x, z, trace=False):
    """Returns (loss, BassKernelResults)."""
    from concourse.bass_utils import run_bass_kernel_spmd

    global _cached
    if _cached is None:
        _cached = _build()
    nc = _cached
    in_maps = _prep_inputs(x, z)
    res = run_bass_kernel_spmd(nc, in_maps, list(range(NCORES)), trace=trace)
    return _epilogue(res.results), res


def kernel(x, z):
    loss, _ = run_on_device(x, z)
    return loss


# revision 10
# speedup vs baseline: 1.5733x; 1.1311x over previous
"""Bass/Trainium2 kernel for nn_GroundingLoss (symmetric token-level InfoNCE).

Math (matches the jax reference exactly):
    sim[a,b,i,j] = sum_k x[a,i,k] * z[b,j,k]
    S[a,b]       = (1/J) * sum_j  [ sum_i softmax_i(sim[a,b,:,j]) * sim[a,b,:,j] ]
    loss         = mean( logsumexp_a(S) - diag + logsumexp_b(S) - diag )

Sharding: the batch axis of x (a) is split across the 8 cores; every core
computes S[a_local, :] against all of z.

Device layout per core (v7): partitions = (b4, j32) per (b,j)-tile (64 tiles
of 128), free = (i, a) with i major, so the softmax i-reduction sits on the
FREE axis and the PE only does the single sim pass.  Single-tile pipeline
stages with PSUM bufs=4 — the PE->ACT->DVE chain needs depth >= 3 or the PE
stalls waiting for PSUM (measured 6us stalls at depth 2).  Per tile:
  PE   4 matmuls [128,512] (K=256 as 2 accum halves, weights reused)
  ACT  e  = exp(sim - SHIFT)     (sole exp engine)
  DVE  es = e * sim              (sole PSUM-capable vector engine; 1024
       elems at ~1.3ns/elem makes this the ~88us floor of the kernel)
  Pool l1e = e i-half fold       (Pool is slow, ~2.5ns/elem, SBUF-only;
       this 512-elem add is all it can afford)
es ships RAW (bf16, 2KB/part/tile) and e ships half-folded (1KB/part/tile),
both on the otherwise-idle SP hwdge queue (~76us, under the span; 28MB HBM
out of ~330GB/s).  Input loads ride the ACT hwdge queue up front.  The host
does the remaining folds in fp32, divides num/den, averages over j, and
runs the tiny [256,256] logsumexp epilogue (softmax weights are
shift-invariant, so no SHIFT correction is needed).
"""

import numpy as np

N, I, J, K = 256, 32, 32, 256
NCORES = 8
NL = N // NCORES          # 32 local a's per core
AF = NL * I               # 1024 rhs cols per K-half (i, a) i-major
BJ = N * J                # 8192 (b, j) pairs
NT = BJ // 128            # 64 (b,j)-tiles of 128 partitions
SHIFT = 60.0              # exp shift: safe for |sim| up to ~130

_cached = None


def _build():
    import concourse.bacc as bacc
    import concourse.mybir as mybir
    import concourse.tile as tile

    f32 = mybir.dt.float32
    bf16 = mybir.dt.bfloat16
    AF_T = mybir.ActivationFunctionType

    nc = bacc.Bacc("TRN2", target_bir_lowering=False, debug=False)
    xt_d = nc.dram_tensor("xt", [128, 2 * AF], bf16, kind="ExternalInput").ap()
    zt_d = nc.dram_tensor("zt", [128, 2 * BJ], bf16, kind="ExternalInput").ap()
    es_d = nc.dram_tensor("es", [128, NT, 1024], bf16, kind="ExternalOutput").ap()
    le_d = nc.dram_tensor("le", [128, NT, 512], bf16, kind="ExternalOutput").ap()

    with tile.TileContext(nc) as tc:
        with (
            tc.tile_pool(name="const", bufs=1) as cpool,
            tc.tile_pool(name="psum", bufs=4, space="PSUM") as ppool,
            tc.tile_pool(name="ees", bufs=4) as epool,
            tc.tile_pool(name="l1", bufs=4) as lpool,
        ):
            bias_t = cpool.tile([128, 1], f32)
            nc.gpsimd.memset(bias_t[:], -SHIFT)
            xt = cpool.tile([128, 2 * AF], bf16)
            zt = cpool.tile([128, 2 * BJ], bf16)
            # input loads on the ACT hwdge queue (SP carries the output
            # stream); interleave the K-halves so early tiles are ready fast
            nc.scalar.dma_start(xt[:], xt_d[:, :])
            nq = 8
            for q in range(nq):
                for kc in range(2):
                    sl = slice(kc * BJ + q * (BJ // nq), kc * BJ + (q + 1) * (BJ // nq))
                    nc.scalar.dma_start(zt[:, sl], zt_d[:, sl])

            for t in range(NT):
                sim = ppool.tile([128, 1024], f32, tag="sim")  # (i, a) flat
                for kc in range(2):
                    lhsT = zt[:, kc * BJ + t * 128 : kc * BJ + (t + 1) * 128]
                    for ih in range(2):
                        nc.tensor.matmul(
                            sim[:, ih * 512 : (ih + 1) * 512],
                            lhsT,
                            xt[:, kc * AF + ih * 512 : kc * AF + (ih + 1) * 512],
                            start=(kc == 0),
                            stop=(kc == 1),
                        )
                ees = epool.tile([128, 2, 1024], bf16, tag="ees")  # (e|es, i*a)
                nc.scalar.activation(ees[:, 0], sim[:], AF_T.Exp, bias=bias_t[:], scale=1.0)
                nc.vector.tensor_mul(ees[:, 1], ees[:, 0], sim[:])
                l1e = lpool.tile([128, 512], bf16, tag="l1e")
                nc.gpsimd.tensor_add(l1e[:], ees[:, 0, 0:512], ees[:, 0, 512:1024])
                nc.sync.dma_start(es_d[:, t], ees[:, 1])
                nc.sync.dma_start(le_d[:, t], l1e[:])
    nc.compile()
    return nc


def _prep_inputs(x, z):
    import ml_dtypes

    bf = ml_dtypes.bfloat16
    x = np.ascontiguousarray(x, dtype=np.float32).astype(bf)
    z = np.ascontiguousarray(z, dtype=np.float32).astype(bf)
    # zt[p, kc*BJ + b*J + j] = z[b, j, kc*128 + p]
    zt = z.transpose(2, 0, 1).reshape(K, BJ)
    zt = np.concatenate([zt[0:128], zt[128:256]], axis=1)
    zt = np.ascontiguousarray(zt)
    in_maps = []
    for d in range(NCORES):
        xl = x[d * NL : (d + 1) * NL]                  # [a, i, K]
        xt = xl.transpose(2, 1, 0).reshape(K, AF)      # [K, (i, a)]
        xt = np.concatenate([xt[0:128], xt[128:256]], axis=1)
        in_maps.append({"xt": np.ascontiguousarray(xt), "zt": zt})
    return in_maps


def _epilogue(results):
    S = np.empty((N, N), dtype=np.float64)
    for d in range(NCORES):
        es = results[d]["es"].astype(np.float32).reshape(128, NT, I, NL)
        num = es.sum(axis=2)                           # [(b4,j), t, a]
        le = results[d]["le"].astype(np.float32).reshape(128, NT, 16, NL)
        den = le.sum(axis=2)                           # [(b4,j), t, a]
        r = num / den
        r = r.reshape(4, J, NT, NL).mean(axis=1).astype(np.float64)  # [b4, t, a]
        S[d * NL : (d + 1) * NL, :] = r.transpose(2, 1, 0).reshape(NL, N)
    diag = np.diagonal(S)
    m0 = S.max(axis=0)
    lx = m0 + np.log(np.exp(S - m0[None, :]).sum(axis=0)) - diag
    m1 = S.max(axis=1)
    lz = m1 + np.log(np.exp(S - m1[:, None]).sum(axis=1)) - diag
    loss = (lx + lz).mean()
    return np.asarray(loss, dtype=np.float32)


def run_on_device(x, z, trace=False):
    """Returns (loss, BassKernelResults)."""
    from concourse.bass_utils import run_bass_kernel_spmd

    global _cached
    if _cached is None:
        _cached = _build()
    nc = _cached
    in_maps = _prep_inputs(x, z)
    res = run_bass_kernel_spmd(nc, in_maps, list(range(NCORES)), trace=trace)
    return _epilogue(res.results), res


def kernel(x, z):
    loss, _ = run_on_device(x, z)
    return loss
